# revision 120
# baseline (speedup 1.0000x reference)
"""nGPT-style cosine-norm attention on 8 TRN2 NeuronCores, data-parallel over batch.

Per core (one batch element, tokens N=1024, dim 768, 12 heads x 64):
  qT/kT = WT_eff @ xT  (head-dim on partitions), v in token-major layout (+ones col)
  ss    = blockdiag(1/s_eff^2) @ (qT^2)  -> per-head token norms via PE
  rq,rk = exp(-0.5 ln ss);  qn = qT * bcast(rq) (DMA row-broadcast)
  S^T   = kn_h^T q_h  per (head, jtile);  E = exp(8*rk_j * S^T) (ACT per-partition scale)
  PV    = flipped: out[i(128), 65] = sum_jt E_jt[:, itile]^T [V_jt | 1]
          (M=128 full, free=65 -> half the PE rows of the unflipped form);
          col 64 = softmax denominator; evict = DVE tensor_scalar by 1/denom
  attn  = PE-transpose of token-major tiles back to dim-major chunks
  out   = attn^T @ WoT in staged chunks ((0,1) after pair 1, (2,3)/(4) later,
          chunk 5 + parts inject in the tail) emitted as PE fillers inside the
          ACT-bound attention pairs
Schedule: per chunk-pair, S j-tiles stream with filler work (projections for
pair c+2, out-proj stages) pumped between them so PE never idles at the
exp-paced S-psum rotation; q/k norm stats run two pairs ahead so their ACT
Log/Exp chain and rq DMA-broadcast roundtrip stay off the critical path.
All matmuls bf16 (inputs pre-cast on host), stats/softmax f32, output bf16.
"""
import json
import math

import numpy as np
import ml_dtypes

B, N, DIM, H, HD = 8, 1024, 768, 12, 64
P = 128
CH = DIM // P  # 6 chunks of 128 rows; chunk c holds heads 2c, 2c+1
SCALE = float(math.sqrt(HD))
BF = ml_dtypes.bfloat16

_cache = {}


def _split_waits(nc, cap=1):
    """This walrus build caps sync-waits per instruction (1 for several structs).
    Move excess waits onto NoOps inserted immediately before, same engine."""
    from bass_rust import module_from_json_bytes

    js = json.loads(nc.to_json_bytes())
    ctr = 0
    for f in js["functions"]:
        for bb in f["blocks"]:
            newl = []
            for inst in bb["instructions"]:
                si = inst.get("sync_info")
                waits = (si or {}).get("on_wait") or []
                if len(waits) > cap:
                    extra, keep = waits[:-cap], waits[-cap:]
                    for k in range(0, len(extra), cap):
                        ctr += 1
                        newl.append({
                            "debug": inst.get("debug", 0),
                            "engine": inst["engine"],
                            "ins": [], "outs": [],
                            "name": f"wsplit-{ctr}",
                            "opcode": "NoOp",
                            "sync_info": {"on_update": [],
                                          "on_wait": extra[k:k + cap]},
                        })
                    si["on_wait"] = keep
                newl.append(inst)
            bb["instructions"] = newl
    nc.m = module_from_json_bytes(json.dumps(js).encode())


def build_nc(repeat=1):
    import concourse.bass as bass
    import concourse.tile as tile
    from concourse import mybir

    f32 = mybir.dt.float32
    bf16 = mybir.dt.bfloat16
    Exp = mybir.ActivationFunctionType.Exp
    Log = mybir.ActivationFunctionType.Ln
    Copy = mybir.ActivationFunctionType.Copy
    mult = mybir.AluOpType.mult
    add = mybir.AluOpType.add

    nc = bass.Bass("TRN2", num_devices=8)
    xT_d = nc.dram_tensor("xT", [DIM, N], bf16, kind="ExternalInput")
    wq_d = nc.dram_tensor("wq", [DIM, DIM], bf16, kind="ExternalInput")
    wk_d = nc.dram_tensor("wk", [DIM, DIM], bf16, kind="ExternalInput")
    wv_d = nc.dram_tensor("wv", [DIM, DIM], bf16, kind="ExternalInput")
    wo_d = nc.dram_tensor("wo", [DIM, DIM], bf16, kind="ExternalInput")
    invs2_d = nc.dram_tensor("invs2", [P, CH * 4], bf16, kind="ExternalInput")
    identT_d = nc.dram_tensor("identT", [P, P], bf16, kind="ExternalInput")
    out_d = nc.dram_tensor("out", [N, DIM], bf16, kind="ExternalOutput")

    with tile.TileContext(nc) as tc:
        with (
            tc.tile_pool(name="persist", bufs=1) as pp,
            tc.tile_pool(name="dram", bufs=1, space="DRAM") as dp,
            tc.tile_pool(name="epool", bufs=20) as ep,
            tc.tile_pool(name="tmpool", bufs=16) as tmp,
            tc.tile_pool(name="bcast", bufs=2) as bcp,
            tc.tile_pool(name="small", bufs=2) as smp,
            tc.tile_pool(name="rdp", bufs=4) as rdp,
            tc.tile_pool(name="sqp", bufs=3) as sqp,
            tc.tile_pool(name="outp", bufs=4) as outp,
        ):
            xT = pp.tile([P, CH, N], bf16)
            wq = pp.tile([P, CH, DIM], bf16)
            wk = pp.tile([P, CH, DIM], bf16)
            wv = pp.tile([P, CH, DIM], bf16)
            wo = pp.tile([P, CH, DIM], bf16)
            invs2 = pp.tile([P, CH, 4], bf16)
            identT = pp.tile([P, P], bf16)
            qTs = [pp.tile([P, N], bf16, name=f"qT{c}") for c in range(CH)]
            kTs = [pp.tile([P, N], bf16, name=f"kT{c}") for c in range(CH)]
            v1 = pp.tile([P, 8, H, HD + 1], bf16)
            attns = [pp.tile([P, N], bf16, name=f"attn{c}") for c in range(CH)]
            rkT = pp.tile([P, 8, H], f32)

            parts = [pp.tile([P, DIM], bf16, name=f"part{m}") for m in range(8)]
            rq_dram = dp.tile([H, N], bf16)
            rk_dram = dp.tile([H, N], f32)

            for _rep in range(repeat):
                xTr = xT_d[:, :].rearrange("(c p) n -> p c n", p=P)
                wqr = wq_d[:, :].rearrange("(c p) o -> p c o", p=P)
                wkr = wk_d[:, :].rearrange("(c p) o -> p c o", p=P)
                wvr = wv_d[:, :].rearrange("(c p) o -> p c o", p=P)
                wor = wo_d[:, :].rearrange("(c p) o -> p c o", p=P)
                # first vproj matmul needs x chunk 0 + wv chunk 0: issue first
                # DMA priority: x + wv feed the vproj prologue first, then
                # wq/wk for the projections, wo last (needed ~100us in)
                nc.sync.dma_start(out=xT[:, 0, :], in_=xTr[:, 0, :])
                nc.gpsimd.dma_start(out=wv[:, 0, :], in_=wvr[:, 0, :])
                for k in range(1, CH):
                    eng = nc.sync if k % 2 == 0 else nc.scalar
                    eng.dma_start(out=xT[:, k, :], in_=xTr[:, k, :])
                for k in range(1, CH):
                    nc.gpsimd.dma_start(out=wv[:, k, :], in_=wvr[:, k, :])
                for k in range(CH):
                    nc.gpsimd.dma_start(out=wq[:, k, :], in_=wqr[:, k, :])
                for k in range(CH):
                    nc.gpsimd.dma_start(out=wk[:, k, :], in_=wkr[:, k, :])
                nc.scalar.dma_start(out=invs2, in_=invs2_d[:, :].rearrange("p (c h) -> p c h", h=4))
                nc.scalar.dma_start(out=identT, in_=identT_d[:, :])
                for k in range(CH):
                    nc.gpsimd.dma_start(out=wo[:, k, :], in_=wor[:, k, :])

                with (
                    tc.tile_pool(name="sps", bufs=4, space="PSUM") as sps,
                    tc.tile_pool(name="auxps", bufs=2, space="PSUM") as axp,
                    tc.tile_pool(name="pvps", bufs=2, space="PSUM") as pvp,
                ):
                    # v projection (token-major); tile preset to 1.0 so the
                    # 65th column is the softmax-denominator ones column
                    nc.vector.memset(v1[:, :, :, :], 1.0)

                    def emit_vproj(m):
                        for o2, (o0, o1) in enumerate(((0, 512), (512, 768))):
                            ps = axp.tile([P, 512], f32, tag="aux")
                            for k in range(CH):
                                nc.tensor.matmul(
                                    ps[:, 0:o1 - o0],
                                    xT[:, k, m * P:(m + 1) * P],
                                    wv[:, k, o0:o1],
                                    start=(k == 0), stop=(k == CH - 1),
                                )
                            nc.vector.tensor_copy(
                                out=v1[:, m, o0 // HD:o1 // HD, 0:HD],
                                in_=ps[:, 0:o1 - o0].rearrange("p (h d) -> p h d", d=HD),
                            )

                    def emit_vproj_pair(m0, m1):
                        """Two m-tiles' vproj groups k-interleaved across the
                        aux and pv psum pools: 4 matmuls ready per arriving
                        x/wv DMA chunk, so startup is PE-bound not DMA-bound."""
                        halves = ((0, 512), (512, 768))
                        pss = {}
                        for (mm, pool, tag) in ((m0, axp, "aux"), (m1, pvp, "pv")):
                            for o0, o1 in halves:
                                pss[mm, o0] = pool.tile([P, 512], f32, tag=tag,
                                                        name=f"vps{mm}_{o0}")
                        for k in range(CH):
                            for mm in (m0, m1):
                                for o0, o1 in halves:
                                    nc.tensor.matmul(
                                        pss[mm, o0][:, 0:o1 - o0],
                                        xT[:, k, mm * P:(mm + 1) * P],
                                        wv[:, k, o0:o1],
                                        start=(k == 0), stop=(k == CH - 1),
                                    )
                        for mm in (m0, m1):
                            for o0, o1 in halves:
                                nc.vector.tensor_copy(
                                    out=v1[:, mm, o0 // HD:o1 // HD, 0:HD],
                                    in_=pss[mm, o0][:, 0:o1 - o0].rearrange(
                                        "p (h d) -> p h d", d=HD),
                                )

                    def emit_proj_group(c, qk, n2):
                        """One quarter of the q/k projection for chunk c."""
                        dst, w = ((qTs[c], wq), (kTs[c], wk))[qk]
                        nsl = slice(n2 * 512, (n2 + 1) * 512)
                        ps = axp.tile([P, 512], f32, tag="aux")
                        for k in range(CH):
                            nc.tensor.matmul(
                                ps[:, 0:512],
                                w[:, k, c * P:(c + 1) * P],
                                xT[:, k, nsl],
                                start=(k == 0), stop=(k == CH - 1),
                            )
                        nc.vector.tensor_copy(out=dst[:, nsl], in_=ps[:, 0:512])

                    def emit_stats(c, act_drain=False):
                        # token-major stats: ss[token, head] = sq_jt^T @ invs2
                        # (free dim 2, so 16 matmuls cost ~nothing on PE), one
                        # tiny ACT Log per j-tile (the one-accumulation-group-
                        # per-bank rule forces a reader between j-tiles; S-pool
                        # slots are safe since their next S users are a full
                        # pair away). rk lands directly in rkT's token-major
                        # layout - no DRAM roundtrip. invs2's k-columns carry
                        # the 1/64 logit scale.
                        rqt = smp.tile([P, 8, 2], bf16, tag="rqt")
                        eps = smp.tile([P, 1], f32, tag="epst")
                        nc.vector.memset(eps, 1e-12)
                        for si, src in enumerate((qTs[c], kTs[c])):
                            sq = sqp.tile([P, N], bf16, tag="sq")
                            nc.vector.tensor_tensor(sq, src, src, mult)
                            # DVE copies (not ACT Logs) drain each ss bank:
                            # the DVE queue isn't clogged by lagging exps, so
                            # the tiny matmuls never stall at exp pace; one
                            # Log+Exp per src then handles all 16 values
                            if not act_drain:
                                ss16 = smp.tile([P, 16], f32, tag=f"ss16_{si}")
                            lns = smp.tile([P, 16], f32, tag=f"lnt{si}")
                            for jt in range(8):
                                ss = sps.tile([P, 2], f32, tag="S",
                                              name=f"ss{si}_{jt}")
                                nc.tensor.matmul(
                                    ss, sq[:, jt * P:(jt + 1) * P],
                                    invs2[:, c, 2 * si:2 * si + 2],
                                    start=True, stop=True,
                                )
                                if act_drain:
                                    # prologue: ACT is idle, drain via Log
                                    nc.scalar.activation(
                                        out=lns[:, 2 * jt:2 * jt + 2], in_=ss,
                                        func=Log, bias=eps[:, 0:1])
                                else:
                                    nc.vector.tensor_copy(
                                        out=ss16[:, 2 * jt:2 * jt + 2], in_=ss)
                            if not act_drain:
                                nc.scalar.activation(out=lns, in_=ss16,
                                                     func=Log, bias=eps[:, 0:1])
                            if si == 0:
                                nc.scalar.activation(
                                    out=rqt.rearrange("p a b -> p (a b)"),
                                    in_=lns, func=Exp, scale=-0.5)
                            else:
                                nc.scalar.activation(
                                    out=rkT[:, :, 2 * c:2 * c + 2],
                                    in_=lns.rearrange("p (a b) -> p a b", b=2),
                                    func=Exp, scale=-0.5)

                        # broadcast rq rows via DRAM row-broadcast; bf16
                        # multiplier makes the qn multiply a 2x DVE op
                        for hh in range(2):
                            nc.sync.dma_start(
                                out=rq_dram[2 * c + hh, :].rearrange(
                                    "(jt p) -> p jt", p=P),
                                in_=rqt[:, :, hh],
                            )
                        mq = bcp.tile([P, N], bf16, tag="mq")
                        for hh in range(2):
                            row = rq_dram[2 * c + hh:2 * c + hh + 1, :]
                            bc = bass.AP(tensor=row.tensor, offset=row.offset,
                                         ap=[[0, 64]] + list(row.ap[1:]))
                            nc.sync.dma_start(out=mq[hh * 64:(hh + 1) * 64, :], in_=bc)
                        nc.vector.tensor_tensor(qTs[c], qTs[c], mq, mult)

                    def emit_S_half(h, jt, n2, e):
                        c, half = h // 2, (h % 2) * 64
                        nsl = slice(n2 * 512, (n2 + 1) * 512)
                        s = sps.tile([P, 512], f32, tag="S")
                        nc.tensor.matmul(
                            s,
                            kTs[c][half:half + 64, jt * P:(jt + 1) * P],
                            qTs[c][half:half + 64, nsl],
                            start=True, stop=True,
                        )
                        nc.scalar.activation(out=e[:, nsl], in_=s, func=Exp,
                                             scale=rkT[:, jt, h:h + 1])

                    def emit_S_jt(h, jt, e):
                        """One j-tile of S^T + exp into E tile e [128 j, 1024 i]."""
                        for n2 in range(2):
                            emit_S_half(h, jt, n2, e)

                    def emit_PV_it(h, Es, tms, it):
                        """Flipped PV: out[i(128), V|1(65)] accumulated over jt;
                        denominator in col 64; evict scaled by 1/denom."""
                        half = (h % 2) * 64
                        pv = pvp.tile([P, HD + 1], f32, tag="pv")
                        for jt in range(8):
                            nc.tensor.matmul(
                                pv, Es[jt][:, it * P:(it + 1) * P],
                                v1[:, jt, h, :],
                                start=(jt == 0), stop=(jt == 7),
                            )
                        rd = rdp.tile([P, 1], f32, tag="rd")
                        nc.vector.reciprocal(rd, pv[:, HD:HD + 1])
                        nc.vector.tensor_scalar_mul(
                            tms[it][:, half:half + 64], pv[:, 0:HD],
                            rd[:, 0:1])

                    def emit_transpose_it(c, tms, it):
                        # rides the pv pool: a [128,128] bf16 tile fits the
                        # [128,65] f32 slot, so this costs no PSUM banks and
                        # keeps transposes out of the contended aux rotation
                        tp = pvp.tile([P, P], bf16, tag="pv")
                        nc.tensor.matmul(tp, tms[it], identT, is_transpose=True)
                        nc.vector.tensor_copy(
                            out=attns[c][:, it * P:(it + 1) * P], in_=tp)

                    osb2_st = {}

                    def emit_outproj(m, cs, final):
                        """Accumulate chunks cs of the output projection for
                        m-tile m into parts[m] (or emit final add + DMA).
                        The final stage borrows the (by then idle) S psum
                        pool so psum rotation never waits on the adds."""
                        pool, tag = (sps, "S") if final else (axp, "aux")
                        pss = []
                        for o0, o1 in ((0, 512), (512, 768)):
                            ps = pool.tile([P, 512], f32, tag=tag)
                            # in the final stage the 256-half (and for the
                            # last m-tiles both halves) absorbs parts[m] via an
                            # identity-matmul inject so eviction is a plain
                            # ACT Copy (no DVE add on the tail critical path)
                            inject = final and o0 == 512
                            for i, c in enumerate(cs):
                                nc.tensor.matmul(
                                    ps[:, 0:o1 - o0],
                                    attns[c][:, m * P:(m + 1) * P],
                                    wo[:, c, o0:o1],
                                    start=(i == 0),
                                    stop=(i == len(cs) - 1) and not inject,
                                )
                            if inject:
                                nc.tensor.matmul(
                                    ps[:, 0:o1 - o0], identT,
                                    parts[m][:, o0:o1],
                                    start=False, stop=True,
                                )
                            pss.append(ps)
                        if not final:
                            first = cs[0] == 0
                            for (o0, o1), ps in zip(((0, 512), (512, 768)), pss):
                                if first:
                                    nc.vector.tensor_copy(out=parts[m][:, o0:o1],
                                                          in_=ps[:, 0:o1 - o0])
                                else:
                                    nc.vector.tensor_tensor(
                                        parts[m][:, o0:o1], ps[:, 0:o1 - o0],
                                        parts[m][:, o0:o1], add)
                        else:
                            # adjacent m-tiles share one osb tile and go
                            # out as a single DMA (m-blocks are contiguous in
                            # DRAM), halving HWDGE's per-DMA fixed cost in
                            # the drain
                            if m % 2 == 0:
                                osb2_st["t"] = outp.tile([P, 2, DIM], bf16,
                                                         tag="osb",
                                                         name=f"osb{m}")
                            osb = osb2_st["t"][:, m % 2, :]
                            nc.vector.tensor_tensor(
                                osb[:, 0:512], pss[0][:, 0:512],
                                parts[m][:, 0:512], add)
                            nc.scalar.activation(out=osb[:, 512:768],
                                                 in_=pss[1][:, 0:256], func=Copy)
                            # last pair goes out as singles: latency beats
                            # HWDGE overhead at the very end of the drain;
                            # the very last m-tile goes in halves so its
                            # final (small) transfer+sem chain starts sooner
                            if m == 6:
                                nc.sync.dma_start(
                                    out=out_d[m * P:(m + 1) * P, :], in_=osb)
                            elif m == 7:
                                nc.sync.dma_start(
                                    out=out_d[m * P:(m + 1) * P, 0:512],
                                    in_=osb[:, 0:512])
                                nc.scalar.dma_start(
                                    out=out_d[m * P:(m + 1) * P, 512:768],
                                    in_=osb[:, 512:768])
                            elif m % 2 == 1:
                                eng = nc.sync if m % 4 == 1 else nc.scalar
                                eng.dma_start(
                                    out=out_d[(m - 1) * P:(m + 1) * P, :]
                                    .rearrange("(b p) d -> p b d", p=P),
                                    in_=osb2_st["t"])

                    # prologue: v projection, then q/k proj+stats for pairs
                    # 0 and 1 (stats run two pairs ahead from here on so the
                    # ACT Log/Exp chain and rq/rk DMA roundtrips never sit in
                    # the exp-critical window)
                    emit_vproj_pair(0, 1)
                    emit_vproj_pair(2, 3)
                    for m in (4, 5):
                        emit_vproj(m)
                    for c in (0, 1):
                        for qk in range(2):
                            for n2 in range(2):
                                emit_proj_group(c, qk, n2)
                        emit_stats(c, act_drain=True)

                    for c in range(CH):
                        h0, h1 = 2 * c, 2 * c + 1
                        tms = [tmp.tile([P, P], bf16, tag="tm", name=f"tm{c}_{it}")
                               for it in range(8)]
                        Es0 = [ep.tile([P, N], bf16, tag="E", name=f"E{h0}_{jt}")
                               for jt in range(8)]
                        Es1 = [ep.tile([P, N], bf16, tag="E", name=f"E{h1}_{jt}")
                               for jt in range(8)]
                        # fillers: PE work interleaved between S j-tiles so PE
                        # stays busy while ACT drains the exp stream
                        fillers = []
                        if c == 0:
                            for m in (6, 7):
                                fillers.append((lambda m2=m: emit_vproj(m2)))
                        if c + 2 < CH:
                            for qk in range(2):
                                for n2 in range(2):
                                    fillers.append(
                                        (lambda c2=c + 2, qk2=qk, n22=n2:
                                         emit_proj_group(c2, qk2, n22)))
                        # out-projection stages as soon as their chunks exist:
                        # chunks (0,1) land in pairs 2-3, (2,3) in pair 4,
                        # (4,) in pair 5, chunk 5 + final add in the tail
                        if c == 2:
                            for m in range(4):
                                fillers.append(
                                    (lambda m2=m: emit_outproj(m2, [0, 1],
                                                               final=False)))
                        if c == 3:
                            for m in range(4, 8):
                                fillers.append(
                                    (lambda m2=m: emit_outproj(m2, [0, 1],
                                                               final=False)))
                        if c == 4:
                            for m in range(4):
                                fillers.append(
                                    (lambda m2=m: emit_outproj(m2, [2, 3],
                                                               final=False)))
                        if c == 5:
                            for m in range(4, 8):
                                fillers.append(
                                    (lambda m2=m: emit_outproj(m2, [2, 3],
                                                               final=False)))
                            for m in range(8):
                                fillers.append(
                                    (lambda m2=m: emit_outproj(m2, [4],
                                                               final=False)))

                        def pump():
                            if fillers:
                                fillers.pop(0)()

                        # phase A: S(h0) j-tiles with fillers interleaved;
                        # the last pair holds some fillers back for phase C
                        pumpA = (1,) if c == CH - 1 else (1, 3, 5, 7)
                        pumpB = (0, 3) if c == CH - 1 else (0, 2, 4, 6)
                        for jt in range(8):
                            emit_S_jt(h0, jt, Es0[jt])
                            if jt in pumpA:
                                pump()
                        # phase B: S(h1) j-tiles + PV(h0) staggered. The
                        # last head goes half-major (all first halves, then
                        # all second halves): PV(h1, it 0-3) only read E's
                        # first halves, so their gate moves ~8 exp-halves
                        # earlier and the post-exp tail chain shortens.
                        if c == CH - 1:
                            seq = [(jt, 0) for jt in range(8)] + \
                                  [(jt, 1) for jt in range(8)]
                        else:
                            seq = [(jt, n2) for jt in range(8)
                                   for n2 in range(2)]
                        for step, (jt, n2) in enumerate(seq):
                            emit_S_half(h1, jt, n2, Es1[jt])
                            if step % 2 == 1:
                                sjt = step // 2
                                if sjt >= 3:
                                    emit_PV_it(h0, Es0, tms, sjt - 3)
                                if sjt in pumpB:
                                    pump()
                        # drain any leftover proj fillers BEFORE stats: the
                        # stats sq op reads qTs[c+2] on the same in-order DVE
                        # queue as the proj evictions that produce it
                        if c + 2 < CH:
                            while fillers:
                                pump()
                        for it in (5, 6, 7):
                            emit_PV_it(h0, Es0, tms, it)
                            pump()
                        while fillers:
                            pump()
                        # phase C: PV(h1) + transposes staggered; in the last
                        # pair the final out-proj stage (chunk 5 + add + DMA)
                        # chases the transpose stream so output DMAs overlap
                        last = c == CH - 1
                        for it in range(8):
                            emit_PV_it(h1, Es1, tms, it)
                            if it >= 2:
                                emit_transpose_it(c, tms, it - 2)
                            if last and it >= 3:
                                emit_outproj(it - 3, [5], final=True)
                        for it in (6, 7):
                            emit_transpose_it(c, tms, it)
                            if last:
                                emit_outproj(it - 1, [5], final=True)
                        if last:
                            emit_outproj(7, [5], final=True)
                        if c + 2 < CH:
                            emit_stats(c + 2)

    _split_waits(nc, cap=1)
    return nc


def _host_inputs(x, Wq, Wk, Wv, Wo, s_qk):
    s_eff = (np.asarray(s_qk, np.float32).reshape(-1) * math.sqrt(DIM)).astype(np.float32)
    wq = np.ascontiguousarray((s_eff[:, None] * np.asarray(Wq, np.float32)).T).astype(BF)
    wk = np.ascontiguousarray((s_eff[:, None] * np.asarray(Wk, np.float32)).T).astype(BF)
    wv = np.ascontiguousarray(np.asarray(Wv, np.float32).T).astype(BF)
    wo = np.ascontiguousarray(np.asarray(Wo, np.float32).T).astype(BF)
    invs2 = np.zeros((P, CH * 4), np.float32)
    for o in range(DIM):
        c, p = o // P, o % P
        hh = p // HD  # head within chunk (0 or 1)
        invs2[p, c * 4 + hh] = 1.0 / (s_eff[o] * s_eff[o])
        invs2[p, c * 4 + 2 + hh] = 1.0 / (HD * s_eff[o] * s_eff[o])
    invs2 = invs2.astype(BF)
    identT = np.eye(P, dtype=np.float32).astype(BF)
    shared = dict(wq=wq, wk=wk, wv=wv, wo=wo, invs2=invs2, identT=identT)
    in_maps = []
    for b in range(B):
        m = dict(shared)
        m["xT"] = np.ascontiguousarray(np.asarray(x[b], np.float32).T).astype(BF)
        in_maps.append(m)
    return in_maps


def run(x, Wq, Wk, Wv, Wo, s_qk, trace=False, **trace_kwargs):
    from concourse.bass_utils import run_bass_kernel_spmd

    if "nc" not in _cache:
        _cache["nc"] = build_nc()
    nc = _cache["nc"]
    in_maps = _host_inputs(x, Wq, Wk, Wv, Wo, s_qk)
    res = run_bass_kernel_spmd(nc, in_maps, core_ids=list(range(8)),
                               trace=trace, **trace_kwargs)
    out = np.stack([res.results[b]["out"] for b in range(B)]).astype(np.float32)
    return out, res


def kernel(x, Wq, Wk, Wv, Wo, s_qk):
    out, _ = run(x, Wq, Wk, Wv, Wo, s_qk, trace=False)
    return out
